# revision 15
# baseline (speedup 1.0000x reference)
"""Trainium2 Bass kernel for nn_BoundaryLoss (8-core SPMD).

Strategy
--------
Shard the label axis (K=150, padded to 152 = 8*19) across the 8 cores.
Per core, for each of its 19 labels k:
    dist2[b, :] accumulation:  psum[b_chunk, i] += oodT[j, b_chunk].T @ R_k.T[j, i]
  (out layout [b, i] so the norm reduction is a free-axis reduce), then a
  fused Square+accumulate (ScalarE) produces dist^2 per (label, b_chunk).
The positive term rides along as a 5th "b_chunk" whose stationary operand is
the core's own gathered (pooled - centroid[label]) columns; a host-built mask
selects the (slot, label) pairs that are real.
A tiny batched epilogue (sqrt/exp/relu/sign + masked reduces) turns dist^2
into the four partial sums [pos_loss_sum, neg_loss_sum(masked per-label),
pos_num, neg_num] per partition; the host sums partitions and cores and forms
the 5 scalar outputs.

Matmuls run as float32r (full-rate fp32) with N=512/256 moving tiles.
"""

import math

import numpy as np

try:
    import concourse.bacc as bacc
    import concourse.mybir as mybir
    import concourse.tile as tile
    from concourse.bass_utils import run_bass_kernel_spmd
except ImportError:  # pragma: no cover - fallback for bare environments
    import sys

    sys.path.insert(0, "/opt/trn_rl_repo")
    import concourse.bacc as bacc
    import concourse.mybir as mybir
    import concourse.tile as tile
    from concourse.bass_utils import run_bass_kernel_spmd

K = 150
D = 768
B = 512
BETA = 0.3
NCORES = 8
KPC = 19                # labels per core
KPAD = NCORES * KPC     # 152
NJ = D // 128           # 6 contraction chunks
NBC = 5                 # 4 ood b-chunks + 1 pos chunk
NCOL = KPC * NBC        # 95 accumulator columns per core
F32 = mybir.dt.float32

_prog_cache = {}


def build_program(mode="f32r", debug_acc=False, warmup=16, tag=None,
                  rt_split_labels=KPC, rt_bufs=4):
    """Build the SPMD bass program. mode in {"f32r", "f32", "bf16"}."""
    store_dt = {"f32r": mybir.dt.float32r, "f32": F32,
                "bf16": mybir.dt.bfloat16}[mode]
    nc = bacc.Bacc("TRN2", target_bir_lowering=False, debug=False,
                   num_devices=NCORES)
    if tag is not None:
        nc.dram_tensor(f"tag_{tag}", [1, 1], F32, kind="ExternalInput")
    accdump = None
    if debug_acc:
        accdump = nc.dram_tensor("accdump", [128, NCOL], F32,
                                 kind="ExternalOutput").ap()

    rt = nc.dram_tensor("rt", [KPC, 128, NJ * D], store_dt,
                        kind="ExternalInput").ap()
    oodt = nc.dram_tensor("oodt", [128, NJ * B], store_dt,
                          kind="ExternalInput").ap()
    ck = nc.dram_tensor("ck", [128, KPC * NJ], F32,
                        kind="ExternalInput").ap()
    xpost = nc.dram_tensor("xpost", [128, NJ * 128], store_dt,
                           kind="ExternalInput").ap()
    dk = nc.dram_tensor("dk", [128, NCOL], F32, kind="ExternalInput").ap()
    oodmask = nc.dram_tensor("oodmask", [128, NCOL], F32,
                             kind="ExternalInput").ap()
    posmask = nc.dram_tensor("posmask", [128, NCOL], F32,
                             kind="ExternalInput").ap()
    partials = nc.dram_tensor("partials", [128, 4], F32,
                              kind="ExternalOutput").ap()

    def mmcast(ap):
        return ap

    # Register the Exp bias constant (ln BETA) the way Bass.__init__ does.
    LNBETA = float(math.log(BETA))
    const_t = nc.alloc_sbuf_tensor("const-lnbeta", [128, 1], F32)
    nc.gpsimd.memset(const_t.ap(), LNBETA)
    nc.const_aps.aps[(F32, LNBETA)] = const_t.ap()
    nc.all_engine_barrier()

    AF = mybir.ActivationFunctionType
    ALU = mybir.AluOpType

    with tile.TileContext(nc) as tc:
        with (
            tc.tile_pool(name="consts", bufs=1) as cpool,
            tc.tile_pool(name="rtp", bufs=rt_bufs) as rtpool,
            tc.tile_pool(name="scratch", bufs=3) as spool,
            tc.tile_pool(name="psum", bufs=4, space="PSUM") as ppool,
            tc.tile_pool(name="fin", bufs=1) as fpool,
        ):
            # Load order matters for the critical path: oodt+ck feed the
            # first oodc subtraction; xpost is first needed at bc==4 of
            # label 0; dk/masks only at the final epilogue.
            ck_s = cpool.tile([128, KPC * NJ], F32)
            nc.sync.dma_start(out=ck_s[:], in_=ck[:])
            oodt_s = cpool.tile([128, NJ * B], store_dt)
            for jc in range(NJ):
                nc.sync.dma_start(out=oodt_s[:, jc * B:(jc + 1) * B],
                                  in_=oodt[:, jc * B:(jc + 1) * B])
            xpost_s = cpool.tile([128, NJ * 128], store_dt)
            nc.sync.dma_start(out=xpost_s[:], in_=xpost[:])
            dk_s = cpool.tile([128, NCOL], F32)
            om_s = cpool.tile([128, NCOL], F32)
            pm_s = cpool.tile([128, NCOL], F32)
            acc = cpool.tile([128, NCOL], F32)

            # Optional HAM warm-up: dummy matmuls on the (already loaded)
            # oodt tile while the first rt DMA is in flight. These have no
            # data deps on rt, so Tile schedules them first; they warm the
            # PE clock gate so the real stream runs at 2.4 GHz sooner.
            for w in range(warmup):
                wps = ppool.tile([128, 1024], F32, tag="ps")
                nc.tensor.matmul(wps[:, 0:512],
                                 oodt_s[:, 0:128], oodt_s[:, 0:512],
                                 start=True, stop=True)

            for k in range(KPC):
                rtk = rtpool.tile([128, NJ * D], store_dt)
                if k < rt_split_labels:
                    # split the load so this label's first matmuls can
                    # start after ~1/6 of the transfer
                    for jc in range(NJ):
                        nc.sync.dma_start(
                            out=rtk[:, jc * D:(jc + 1) * D],
                            in_=rt[k, :, jc * D:(jc + 1) * D])
                else:
                    nc.sync.dma_start(out=rtk[:], in_=rt[k, :, :])
                # oodc = oodT - c_k (c_k is a per-partition scalar per jc)
                oodc = spool.tile([128, NJ * B], store_dt, tag="oodc")
                for jc in range(NJ):
                    nc.vector.tensor_scalar_sub(
                        oodc[:, jc * B:(jc + 1) * B],
                        oodt_s[:, jc * B:(jc + 1) * B],
                        ck_s[:, k * NJ + jc:k * NJ + jc + 1])
                for bc in range(NBC):
                    ps = ppool.tile([128, 1024], F32, tag="ps")
                    for jc in range(NJ):
                        if bc < 4:
                            lhsT = oodc[:, jc * B + bc * 128:
                                        jc * B + (bc + 1) * 128]
                        else:
                            lhsT = xpost_s[:, jc * 128:(jc + 1) * 128]
                        nc.tensor.matmul(
                            ps[:, 0:512], mmcast(lhsT),
                            mmcast(rtk[:, jc * D:jc * D + 512]),
                            start=(jc == 0), stop=(jc == NJ - 1))
                        nc.tensor.matmul(
                            ps[:, 512:768], mmcast(lhsT),
                            mmcast(rtk[:, jc * D + 512:(jc + 1) * D]),
                            start=(jc == 0), stop=(jc == NJ - 1))
                    sq = spool.tile([128, D], F32)
                    col = k * NBC + bc
                    nc.scalar.activation(sq[:], ps[:, 0:768], AF.Square,
                                         accum_out=acc[:, col:col + 1])

            nc.sync.dma_start(out=dk_s[:], in_=dk[:])
            nc.sync.dma_start(out=om_s[:], in_=oodmask[:])
            nc.sync.dma_start(out=pm_s[:], in_=posmask[:])

            if debug_acc:
                nc.sync.dma_start(out=accdump[:], in_=acc[:])
            # ---- batched epilogue over the [128, 95] accumulator ----
            dist = fpool.tile([128, NCOL], F32)
            nc.scalar.activation(dist[:], acc[:], AF.Sqrt)
            t = fpool.tile([128, NCOL], F32)        # t = dist - dk
            nc.vector.tensor_tensor(out=t[:], in0=dist[:], in1=dk_s[:],
                                    op=ALU.subtract)
            relu_t = fpool.tile([128, NCOL], F32)   # (euc - d)+
            nc.scalar.activation(relu_t[:], t[:], AF.Relu)
            nrelu = fpool.tile([128, NCOL], F32)    # (d - euc)+
            nc.scalar.activation(nrelu[:], t[:], AF.Relu, scale=-1.0)
            m = fpool.tile([128, NCOL], F32)        # 1[d > euc]
            nc.scalar.activation(m[:], nrelu[:], AF.Sign)
            pnum_i = fpool.tile([128, NCOL], F32)   # 1[euc > d]
            nc.scalar.activation(pnum_i[:], relu_t[:], AF.Sign)
            e = fpool.tile([128, NCOL], F32)        # beta * exp(dk - dist)
            nc.scalar.activation(e[:], t[:], AF.Exp, scale=-1.0,
                                 bias=LNBETA)
            # e is only selected when dk <= dist, where e <= beta; clamp so
            # the branchless blend below can't catastrophically cancel.
            nc.vector.tensor_scalar_min(e[:], e[:], BETA)
            inb = fpool.tile([128, NCOL], F32)      # dk - dist + beta
            nc.scalar.activation(inb[:], t[:], AF.Copy, scale=-1.0,
                                 bias=BETA)
            # pl = e + m * (inb - e)   (branchless where(dk > dist, inb, e))
            d1 = fpool.tile([128, NCOL], F32)
            nc.vector.tensor_tensor(out=d1[:], in0=inb[:], in1=e[:],
                                    op=ALU.subtract)
            d2 = fpool.tile([128, NCOL], F32)
            nc.vector.tensor_tensor(out=d2[:], in0=m[:], in1=d1[:],
                                    op=ALU.mult)
            pl = fpool.tile([128, NCOL], F32)
            nc.vector.tensor_tensor(out=pl[:], in0=e[:], in1=d2[:],
                                    op=ALU.add)

            out4 = fpool.tile([128, 4], F32)
            for idx, (a, b) in enumerate([(pl, om_s), (relu_t, pm_s),
                                          (pnum_i, pm_s), (m, pm_s)]):
                tmp = fpool.tile([128, NCOL], F32, tag="redtmp")
                nc.vector.tensor_tensor(out=tmp[:], in0=a[:], in1=b[:],
                                        op=ALU.mult)
                nc.vector.tensor_reduce(out=out4[:, idx:idx + 1], in_=tmp[:],
                                        axis=mybir.AxisListType.X, op=ALU.add)
            nc.sync.dma_start(out=partials[:], in_=out4[:])

    nc.compile()
    return nc


SF = 2048.0     # fp8 scale for the off-diagonal matrix F = R - diag
SX = 16.0       # fp8 scale for the x operand
PSC = SF * SX   # 32768: psum carries PSC * rx
F8 = mybir.dt.float8e4
BF16 = mybir.dt.bfloat16
EPG = 5         # labels per epilogue batch


def _pin_act_table(patch=True):
    """Make the act-table chooser resolve every function to
    natural_log_exp_and_others (covers Square/Ln/Exp — the only ScalarE
    funcs this program uses), so exactly one ACT_TABLE_LOAD is emitted.
    Set ids are indices into act_info.json, so set order/length is kept
    and only the contents of the other sets are blanked."""
    if not patch:
        return
    orig = bacc.get_activation_tables

    def pinned(arch):
        tabs = orig(arch)
        keep = "natural_log_exp_and_others"
        if keep in tabs:
            for name in tabs:
                if name != keep:
                    tabs[name] = set()
        return tabs

    bacc.get_activation_tables = pinned


def build_program_dr8(warmup=10, rt_bufs=4):
    """fp8 DoubleRow program: psum = SF*SX*(F @ x8) + 32768*(x*D)^T."""
    _pin_act_table()
    nc = bacc.Bacc("TRN2", target_bir_lowering=False, debug=False,
                   num_devices=NCORES)

    rt = nc.dram_tensor("rt", [KPC, 128, NJ * D], F8,
                        kind="ExternalInput").ap()
    oodt = nc.dram_tensor("oodt", [128, NJ * B], BF16,
                          kind="ExternalInput").ap()
    ck = nc.dram_tensor("ck", [128, KPC * NJ], F32,
                        kind="ExternalInput").ap()
    dkd = nc.dram_tensor("dkd", [128, KPC * NJ], F32,
                         kind="ExternalInput").ap()
    xpost_q = nc.dram_tensor("xpost_q", [128, NJ * 128], F8,
                             kind="ExternalInput").ap()
    xpost_b = nc.dram_tensor("xpost_b", [128, NJ * 128], BF16,
                             kind="ExternalInput").ap()
    ident = nc.dram_tensor("ident", [128, 128], BF16,
                           kind="ExternalInput").ap()
    dk = nc.dram_tensor("dk", [128, NCOL], F32, kind="ExternalInput").ap()
    oodmask = nc.dram_tensor("oodmask", [128, NCOL], F32,
                             kind="ExternalInput").ap()
    posmask = nc.dram_tensor("posmask", [128, NCOL], F32,
                             kind="ExternalInput").ap()
    partials = nc.dram_tensor("partials", [128, 4], F32,
                              kind="ExternalOutput").ap()

    LNBETA = float(math.log(BETA))
    LNEPS = 1e-12
    const_t = nc.alloc_sbuf_tensor("const-lnbeta", [128, 1], F32)
    nc.gpsimd.memset(const_t.ap(), LNBETA)
    nc.const_aps.aps[(F32, LNBETA)] = const_t.ap()
    const_e = nc.alloc_sbuf_tensor("const-lneps", [128, 1], F32)
    nc.gpsimd.memset(const_e.ap(), LNEPS)
    nc.const_aps.aps[(F32, LNEPS)] = const_e.ap()
    nc.all_engine_barrier()

    AF = mybir.ActivationFunctionType
    ALU = mybir.AluOpType
    DR = mybir.MatmulPerfMode.DoubleRow
    DESCALE = float(1.0 / (PSC * PSC))   # 2^-30: Sqrt input pre-scale

    with tile.TileContext(nc) as tc:
        with (
            tc.tile_pool(name="consts", bufs=1) as cpool,
            tc.tile_pool(name="rtp", bufs=rt_bufs) as rtpool,
            tc.tile_pool(name="scratch", bufs=3) as spool,
            tc.tile_pool(name="psum", bufs=4, space="PSUM") as ppool,
            tc.tile_pool(name="fin", bufs=1) as fpool,
            tc.tile_pool(name="epi", bufs=2) as epool,
        ):
            # --- constants / shared loads, critical-path first ---
            ident_s = cpool.tile([128, 128], BF16)
            nc.sync.dma_start(out=ident_s[:], in_=ident[:])
            oodt_s = cpool.tile([128, NJ, B], BF16)
            for jc in range(NJ):
                nc.sync.dma_start(out=oodt_s[:, jc, :],
                                  in_=oodt[:, jc * B:(jc + 1) * B])
            ck_s = cpool.tile([128, KPC * NJ], F32)
            nc.sync.dma_start(out=ck_s[:], in_=ck[:])
            dkd_s = cpool.tile([128, KPC * NJ], F32)
            nc.sync.dma_start(out=dkd_s[:], in_=dkd[:])
            # needed only from bc==4 of label 0 / first epilogue: loaded
            # after the first rt labels so rt0 wins the DMA queues
            xq_s = cpool.tile([128, NJ, 128], F8)
            xb_s = cpool.tile([128, NJ, 128], BF16)
            dk_s = cpool.tile([128, NCOL], F32)
            om_s = cpool.tile([128, NCOL], F32)
            pm_s = cpool.tile([128, NCOL], F32)
            # per-epilogue-batch accumulators: avoids WAR serialization
            # between later squares and the batched epilogue reads.
            # Last batch is a single label so the serial tail chain is tiny.
            BSIZES = [5, 5, 5, 3, 1]
            assert sum(BSIZES) == KPC
            BSTART, BATCH_OF, BEND = [], {}, set()
            s = 0
            for g, bsz in enumerate(BSIZES):
                BSTART.append(s)
                for kk in range(s, s + bsz):
                    BATCH_OF[kk] = g
                BEND.add(s + bsz - 1)
                s += bsz
            accs = [cpool.tile([128, bsz * NBC], F32, name=f"accb{g}")
                    for g, bsz in enumerate(BSIZES)]
            st_pl = cpool.tile([128, NCOL], F32)
            st_rl = cpool.tile([128, NCOL], F32)
            st_pn = cpool.tile([128, NCOL], F32)
            st_m = cpool.tile([128, NCOL], F32)

            # HAM warm-up on a memset tile: no DMA dependency, starts the
            # PE activity window as early as possible.
            dummy = cpool.tile([128, 384], BF16)
            nc.vector.memset(dummy[:], 1.0)
            for w in range(warmup):
                wps = ppool.tile([128, 1024], F32, tag="ps")
                nc.tensor.matmul(wps[:, 0:384], dummy[:, 0:128], dummy[:],
                                 start=True, stop=True)

            for k in range(KPC):
                rtk = rtpool.tile([128, NJ, D], F8)
                for pr in range(NJ // 2):
                    nc.sync.dma_start(
                        out=rtk[:, 2 * pr:2 * pr + 2, :],
                        in_=rt[k, :, 2 * pr * D:(2 * pr + 2) * D])
                if k == 0:
                    # after rt0 hits the queues; needed from bc==4 of k==0
                    nc.sync.dma_start(out=xq_s[:], in_=xpost_q[:])
                    nc.sync.dma_start(out=xb_s[:], in_=xpost_b[:])
                elif k == 1:
                    # needed first at the label-4 epilogue
                    nc.sync.dma_start(out=dk_s[:], in_=dk[:])
                    nc.sync.dma_start(out=om_s[:], in_=oodmask[:])
                    nc.sync.dma_start(out=pm_s[:], in_=posmask[:])
                # x8 = e4m3(SX*ood - SX*ck)  per jc chunk (fp8 stationary)
                # on GpSimd to keep DVE for xd + squares
                oq = spool.tile([128, NJ, B], F8, tag="oq")
                xd = spool.tile([128, NJ, NBC * 128], BF16, tag="xd")
                for jc in range(NJ):
                    c1 = ck_s[:, k * NJ + jc:k * NJ + jc + 1]
                    d1 = dkd_s[:, k * NJ + jc:k * NJ + jc + 1]
                    nc.gpsimd.tensor_scalar(
                        out=oq[:, jc, :], in0=oodt_s[:, jc, :],
                        scalar1=c1, scalar2=None, op0=ALU.subtract)
                    # xd = (SX*x) * (D/SX) = x*D  (exact-x diagonal path)
                    nc.vector.tensor_scalar(
                        out=xd[:, jc, 0:B], in0=oodt_s[:, jc, :],
                        scalar1=c1, scalar2=d1,
                        op0=ALU.subtract, op1=ALU.mult)
                    nc.vector.tensor_scalar(
                        out=xd[:, jc, B:B + 128], in0=xb_s[:, jc, :],
                        scalar1=d1, scalar2=None, op0=ALU.mult)
                for bc in range(NBC):
                    ps = ppool.tile([128, 1024], F32, tag="ps")
                    for pr in range(NJ // 2):
                        if bc < 4:
                            lhsT = oq[:, 2 * pr:2 * pr + 2,
                                      bc * 128:(bc + 1) * 128]
                        else:
                            lhsT = xq_s[:, 2 * pr:2 * pr + 2, :]
                        nc.tensor.matmul(
                            ps[:, 0:512], lhsT, rtk[:, 2 * pr:2 * pr + 2, 0:512],
                            start=(pr == 0), stop=False, perf_mode=DR,
                            skip_group_check=True)
                        nc.tensor.matmul(
                            ps[:, 512:768], lhsT,
                            rtk[:, 2 * pr:2 * pr + 2, 512:768],
                            start=(pr == 0), stop=False, perf_mode=DR,
                            skip_group_check=True)
                    # exact diagonal: ps[:, jc*128:(jc+1)*128] += 32768*xd^T
                    for jc in range(NJ):
                        lh2 = xd[:, jc, bc * 128:(bc + 1) * 128]
                        nc.tensor.matmul(
                            ps[:, jc * 128:(jc + 1) * 128], lh2, ident_s[:],
                            start=False, stop=(jc == 3 or jc == NJ - 1),
                            skip_group_check=True)
                    sq = spool.tile([128, D], F32, tag="sq")
                    g = BATCH_OF[k]
                    col = (k - BSTART[g]) * NBC + bc
                    nc.scalar.activation(sq[:], ps[:, 0:768], AF.Square,
                                         accum_out=accs[g][:, col:col + 1])

                # --- incremental epilogue once a batch of labels is done ---
                # ScalarE uses only {Ln, Exp, Square}: one act-table set
                # (natural_log_exp_and_others), so no table reloads.
                if k in BEND:
                    g = BATCH_OF[k]
                    g0 = BSTART[g] * NBC
                    w = (k + 1 - BSTART[g]) * NBC
                    sl = slice(g0, g0 + w)
                    # dist = exp(0.5*ln(acc*DESCALE + eps)) == sqrt
                    lg = epool.tile([128, w], F32, tag="e0")
                    nc.scalar.activation(lg[:], accs[g][:], AF.Ln,
                                         scale=DESCALE, bias=1e-12)
                    dist = epool.tile([128, w], F32, tag="e11")
                    nc.scalar.activation(dist[:], lg[:], AF.Exp, scale=0.5)
                    t = epool.tile([128, w], F32, tag="e1")
                    nc.vector.tensor_tensor(out=t[:], in0=dist[:],
                                            in1=dk_s[:, sl], op=ALU.subtract)
                    relu_t = epool.tile([128, w], F32, tag="e2")
                    nc.vector.tensor_scalar_max(relu_t[:], t[:], 0.0)
                    m = epool.tile([128, w], F32, tag="e4")
                    nc.vector.tensor_scalar(out=m[:], in0=t[:], scalar1=0.0,
                                            scalar2=None, op0=ALU.is_lt)
                    pnum_i = epool.tile([128, w], F32, tag="e5")
                    nc.vector.tensor_scalar(out=pnum_i[:], in0=t[:],
                                            scalar1=0.0, scalar2=None,
                                            op0=ALU.is_gt)
                    e = epool.tile([128, w], F32, tag="e6")
                    nc.scalar.activation(e[:], t[:], AF.Exp, scale=-1.0,
                                         bias=LNBETA)
                    nc.vector.tensor_scalar_min(e[:], e[:], BETA)
                    inb = epool.tile([128, w], F32, tag="e7")
                    nc.vector.tensor_scalar(out=inb[:], in0=t[:],
                                            scalar1=-1.0, scalar2=BETA,
                                            op0=ALU.mult, op1=ALU.add)
                    d1t = epool.tile([128, w], F32, tag="e8")
                    nc.vector.tensor_tensor(out=d1t[:], in0=inb[:], in1=e[:],
                                            op=ALU.subtract)
                    d2t = epool.tile([128, w], F32, tag="e9")
                    nc.vector.tensor_tensor(out=d2t[:], in0=m[:], in1=d1t[:],
                                            op=ALU.mult)
                    pl = epool.tile([128, w], F32, tag="e10")
                    nc.vector.tensor_tensor(out=pl[:], in0=e[:], in1=d2t[:],
                                            op=ALU.add)
                    nc.vector.tensor_tensor(out=st_pl[:, sl], in0=pl[:],
                                            in1=om_s[:, sl], op=ALU.mult)
                    nc.vector.tensor_tensor(out=st_rl[:, sl], in0=relu_t[:],
                                            in1=pm_s[:, sl], op=ALU.mult)
                    nc.vector.tensor_tensor(out=st_pn[:, sl], in0=pnum_i[:],
                                            in1=pm_s[:, sl], op=ALU.mult)
                    nc.vector.tensor_tensor(out=st_m[:, sl], in0=m[:],
                                            in1=pm_s[:, sl], op=ALU.mult)

            out4 = fpool.tile([128, 4], F32)
            for idx, a in enumerate([st_pl, st_rl, st_pn, st_m]):
                nc.vector.tensor_reduce(out=out4[:, idx:idx + 1], in_=a[:],
                                        axis=mybir.AxisListType.X, op=ALU.add)
            nc.sync.dma_start(out=partials[:], in_=out4[:])

    nc.compile()
    return nc


def prep_inputs_dr8(pooled_output, centroids, delta, L, U, D_diag, ood,
                    labels):
    """Host-side shard prep for the dr8 program."""
    import ml_dtypes
    E4NP = ml_dtypes.float8_e4m3
    BFNP = ml_dtypes.bfloat16

    pooled_output = np.asarray(pooled_output, np.float32)
    centroids = np.asarray(centroids, np.float32)
    delta = np.asarray(delta, np.float32)
    L = np.asarray(L, np.float32)
    U = np.asarray(U, np.float32)
    D_diag = np.asarray(D_diag, np.float32)
    ood = np.asarray(ood, np.float32)
    labels = np.asarray(labels).astype(np.int64)

    # F^T = (R - diag)^T in DMA layout [k, p, jc*768 + i], scaled by SF.
    rows, cols = np.tril_indices(D, -1)
    rt_all = np.zeros((KPAD, 128, NJ * D), np.float32)
    rt_all[:K, rows % 128, (rows // 128) * D + cols] = U
    rt_all[:K, cols % 128, (cols // 128) * D + rows] = L
    rt_all *= SF
    rt8 = rt_all.astype(E4NP)
    del rt_all

    def pack_cols(mat):  # [768, n] -> [128, NJ*n] in (p, (jc, n)) layout
        n = mat.shape[1]
        return (mat.reshape(NJ, 128, n).transpose(1, 0, 2)
                .reshape(128, NJ * n))

    oodt_h = pack_cols(ood.T * SX).astype(BFNP)

    delta_pad = np.zeros(KPAD, np.float32)
    delta_pad[:K] = delta
    cent_pad = np.zeros((KPAD, D), np.float32)
    cent_pad[:K] = centroids
    ddiag_pad = np.zeros((KPAD, D), np.float32)
    ddiag_pad[:K] = D_diag

    ident_h = (np.eye(128, dtype=np.float32) * PSC).astype(BFNP)

    in_maps = []
    for mcore in range(NCORES):
        k0 = mcore * KPC
        sel = np.where((labels >= k0) & (labels < k0 + KPC))[0]
        sel = sel[np.argsort(labels[sel], kind="stable")]
        n_pos = len(sel)
        assert n_pos <= 128, f"core {mcore} has {n_pos} positive samples"

        xpos = np.zeros((D, 128), np.float32)
        slot_label = np.full(128, -1, np.int64)
        if n_pos:
            xpos[:, :n_pos] = (pooled_output[sel] - centroids[labels[sel]]).T
            slot_label[:n_pos] = labels[sel]
        xpos_sc = pack_cols(xpos * SX)

        dk_t = np.zeros((128, NCOL), np.float32)
        om_t = np.zeros((128, NCOL), np.float32)
        pm_t = np.zeros((128, NCOL), np.float32)
        for kl in range(KPC):
            kg = k0 + kl
            dk_t[:, kl * NBC:(kl + 1) * NBC] = delta_pad[kg]
            if kg < K:
                om_t[:, kl * NBC:kl * NBC + 4] = 1.0
                pm_t[:, kl * NBC + 4] = (slot_label == kg).astype(np.float32)

        ck_t = (cent_pad[k0:k0 + KPC].reshape(KPC, NJ, 128)
                .transpose(2, 0, 1).reshape(128, KPC * NJ)) * SX
        dkd_t = (ddiag_pad[k0:k0 + KPC].reshape(KPC, NJ, 128)
                 .transpose(2, 0, 1).reshape(128, KPC * NJ)) / SX

        in_maps.append({
            "rt": rt8[k0:k0 + KPC],
            "oodt": oodt_h,
            "ck": ck_t.astype(np.float32),
            "dkd": dkd_t.astype(np.float32),
            "xpost_q": xpos_sc.astype(E4NP),
            "xpost_b": xpos_sc.astype(BFNP),
            "ident": ident_h,
            "dk": dk_t,
            "oodmask": om_t,
            "posmask": pm_t,
        })
    return in_maps


def prep_inputs(pooled_output, centroids, delta, L, U, D_diag, ood, labels,
                mode="f32r"):
    """Host-side shard prep. Returns in_maps (list of 8 dicts)."""
    pooled_output = np.asarray(pooled_output, np.float32)
    centroids = np.asarray(centroids, np.float32)
    delta = np.asarray(delta, np.float32)
    L = np.asarray(L, np.float32)
    U = np.asarray(U, np.float32)
    D_diag = np.asarray(D_diag, np.float32)
    ood = np.asarray(ood, np.float32)
    labels = np.asarray(labels).astype(np.int64)

    store_np = np.float32
    if mode == "bf16":
        import ml_dtypes
        store_np = ml_dtypes.bfloat16

    # RT[k] = R[k].T built directly in the DMA layout [k, p, jc*768 + i]:
    # element (row a, col b) of R.T lives at partition a%128, free (a//128)*768+b.
    rows, cols = np.tril_indices(D, -1)
    rt_all = np.zeros((KPAD, 128, NJ * D), np.float32)
    # strict lower of R.T (a=rows, b=cols) holds U
    rt_all[:K, rows % 128, (rows // 128) * D + cols] = U
    # strict upper of R.T (a=cols, b=rows) holds L
    rt_all[:K, cols % 128, (cols // 128) * D + rows] = L
    dia = np.arange(D)
    rt_all[:K, dia % 128, (dia // 128) * D + dia] = D_diag

    def pack_cols(mat):  # [768, n] -> [128, NJ*n] in (p, (jc, n)) layout
        n = mat.shape[1]
        return (mat.reshape(NJ, 128, n).transpose(1, 0, 2)
                .reshape(128, NJ * n).astype(store_np))

    oodt_h = pack_cols(ood.T.astype(np.float32))

    delta_pad = np.zeros(KPAD, np.float32)
    delta_pad[:K] = delta
    cent_pad = np.zeros((KPAD, D), np.float32)
    cent_pad[:K] = centroids

    in_maps = []
    for mcore in range(NCORES):
        k0 = mcore * KPC
        lab_lo, lab_hi = k0, k0 + KPC
        sel = np.where((labels >= lab_lo) & (labels < lab_hi))[0]
        sel = sel[np.argsort(labels[sel], kind="stable")]
        n_pos = len(sel)
        assert n_pos <= 128, f"core {mcore} has {n_pos} positive samples"

        xpos = np.zeros((D, 128), np.float32)
        slot_label = np.full(128, -1, np.int64)
        if n_pos:
            xpos[:, :n_pos] = (pooled_output[sel] - centroids[labels[sel]]).T
            slot_label[:n_pos] = labels[sel]

        dk_t = np.zeros((128, NCOL), np.float32)
        om_t = np.zeros((128, NCOL), np.float32)
        pm_t = np.zeros((128, NCOL), np.float32)
        for kl in range(KPC):
            kg = k0 + kl
            dk_t[:, kl * NBC:(kl + 1) * NBC] = delta_pad[kg]
            if kg < K:
                om_t[:, kl * NBC:kl * NBC + 4] = 1.0
                pm_t[:, kl * NBC + 4] = (slot_label == kg).astype(np.float32)

        ck_t = (cent_pad[k0:k0 + KPC].reshape(KPC, NJ, 128)
                .transpose(2, 0, 1).reshape(128, KPC * NJ).astype(store_np))

        rt_m = rt_all[k0:k0 + KPC]
        if rt_m.dtype != store_np:
            rt_m = rt_m.astype(store_np)
        in_maps.append({
            "rt": rt_m,
            "oodt": oodt_h,
            "ck": ck_t,
            "xpost": pack_cols(xpos),
            "dk": dk_t,
            "oodmask": om_t,
            "posmask": pm_t,
        })
    return in_maps


def combine(results):
    """Host-side reduction of per-core [128, 4] partials to the 5 outputs."""
    tot = np.zeros(4, np.float64)
    for r in results:
        tot += np.asarray(r["partials"], np.float64).sum(axis=0)
    neg_sum, pos_sum, pos_num, neg_num = tot
    pos_mean = pos_sum / B
    neg_mean = neg_sum / B
    return (np.float32(pos_mean), np.float32(neg_mean),
            np.float32(pos_num), np.float32(neg_num),
            np.float32(pos_mean + neg_mean))


def kernel(pooled_output, centroids, delta, L, U, D_diag, ood, labels,
           mode="dr8", trace=False):
    if mode not in _prog_cache:
        if mode == "dr8":
            _prog_cache[mode] = build_program_dr8()
        else:
            _prog_cache[mode] = build_program(mode)
    nc = _prog_cache[mode]
    if mode == "dr8":
        in_maps = prep_inputs_dr8(pooled_output, centroids, delta, L, U,
                                  D_diag, ood, labels)
    else:
        in_maps = prep_inputs(pooled_output, centroids, delta, L, U, D_diag,
                              ood, labels, mode=mode)
    res = run_bass_kernel_spmd(nc, in_maps, list(range(NCORES)), trace=trace)
    out = combine(res.results)
    if trace:
        return out, res
    return out



# revision 16
# speedup vs baseline: 7.0606x; 7.0606x over previous
"""Trainium2 Bass kernel for nn_BoundaryLoss (8-core SPMD).

Strategy
--------
Shard the label axis (K=150, padded to 152 = 8*19) across the 8 cores.
Per core, for each of its 19 labels k:
    dist2[b, :] accumulation:  psum[b_chunk, i] += oodT[j, b_chunk].T @ R_k.T[j, i]
  (out layout [b, i] so the norm reduction is a free-axis reduce), then a
  fused Square+accumulate (ScalarE) produces dist^2 per (label, b_chunk).
The positive term rides along as a 5th "b_chunk" whose stationary operand is
the core's own gathered (pooled - centroid[label]) columns; a host-built mask
selects the (slot, label) pairs that are real.
A tiny batched epilogue (sqrt/exp/relu/sign + masked reduces) turns dist^2
into the four partial sums [pos_loss_sum, neg_loss_sum(masked per-label),
pos_num, neg_num] per partition; the host sums partitions and cores and forms
the 5 scalar outputs.

Matmuls run as float32r (full-rate fp32) with N=512/256 moving tiles.
"""

import math

import numpy as np

try:
    import concourse.bacc as bacc
    import concourse.mybir as mybir
    import concourse.tile as tile
    from concourse.bass_utils import run_bass_kernel_spmd
except ImportError:  # pragma: no cover - fallback for bare environments
    import sys

    sys.path.insert(0, "/opt/trn_rl_repo")
    import concourse.bacc as bacc
    import concourse.mybir as mybir
    import concourse.tile as tile
    from concourse.bass_utils import run_bass_kernel_spmd

K = 150
D = 768
B = 512
BETA = 0.3
NCORES = 8
KPC = 19                # labels per core
KPAD = NCORES * KPC     # 152
NJ = D // 128           # 6 contraction chunks
NBC = 5                 # 4 ood b-chunks + 1 pos chunk
NCOL = KPC * NBC        # 95 accumulator columns per core
F32 = mybir.dt.float32

_prog_cache = {}


def build_program(mode="f32r", debug_acc=False, warmup=16, tag=None,
                  rt_split_labels=KPC, rt_bufs=4):
    """Build the SPMD bass program. mode in {"f32r", "f32", "bf16"}."""
    store_dt = {"f32r": mybir.dt.float32r, "f32": F32,
                "bf16": mybir.dt.bfloat16}[mode]
    nc = bacc.Bacc("TRN2", target_bir_lowering=False, debug=False,
                   num_devices=NCORES)
    if tag is not None:
        nc.dram_tensor(f"tag_{tag}", [1, 1], F32, kind="ExternalInput")
    accdump = None
    if debug_acc:
        accdump = nc.dram_tensor("accdump", [128, NCOL], F32,
                                 kind="ExternalOutput").ap()

    rt = nc.dram_tensor("rt", [KPC, 128, NJ * D], store_dt,
                        kind="ExternalInput").ap()
    oodt = nc.dram_tensor("oodt", [128, NJ * B], store_dt,
                          kind="ExternalInput").ap()
    ck = nc.dram_tensor("ck", [128, KPC * NJ], F32,
                        kind="ExternalInput").ap()
    xpost = nc.dram_tensor("xpost", [128, NJ * 128], store_dt,
                           kind="ExternalInput").ap()
    dk = nc.dram_tensor("dk", [128, NCOL], F32, kind="ExternalInput").ap()
    oodmask = nc.dram_tensor("oodmask", [128, NCOL], F32,
                             kind="ExternalInput").ap()
    posmask = nc.dram_tensor("posmask", [128, NCOL], F32,
                             kind="ExternalInput").ap()
    partials = nc.dram_tensor("partials", [128, 4], F32,
                              kind="ExternalOutput").ap()

    def mmcast(ap):
        return ap

    # Register the Exp bias constant (ln BETA) the way Bass.__init__ does.
    LNBETA = float(math.log(BETA))
    const_t = nc.alloc_sbuf_tensor("const-lnbeta", [128, 1], F32)
    nc.gpsimd.memset(const_t.ap(), LNBETA)
    nc.const_aps.aps[(F32, LNBETA)] = const_t.ap()
    nc.all_engine_barrier()

    AF = mybir.ActivationFunctionType
    ALU = mybir.AluOpType

    with tile.TileContext(nc) as tc:
        with (
            tc.tile_pool(name="consts", bufs=1) as cpool,
            tc.tile_pool(name="rtp", bufs=rt_bufs) as rtpool,
            tc.tile_pool(name="scratch", bufs=3) as spool,
            tc.tile_pool(name="psum", bufs=4, space="PSUM") as ppool,
            tc.tile_pool(name="fin", bufs=1) as fpool,
        ):
            # Load order matters for the critical path: oodt+ck feed the
            # first oodc subtraction; xpost is first needed at bc==4 of
            # label 0; dk/masks only at the final epilogue.
            ck_s = cpool.tile([128, KPC * NJ], F32)
            nc.sync.dma_start(out=ck_s[:], in_=ck[:])
            oodt_s = cpool.tile([128, NJ * B], store_dt)
            for jc in range(NJ):
                nc.sync.dma_start(out=oodt_s[:, jc * B:(jc + 1) * B],
                                  in_=oodt[:, jc * B:(jc + 1) * B])
            xpost_s = cpool.tile([128, NJ * 128], store_dt)
            nc.sync.dma_start(out=xpost_s[:], in_=xpost[:])
            dk_s = cpool.tile([128, NCOL], F32)
            om_s = cpool.tile([128, NCOL], F32)
            pm_s = cpool.tile([128, NCOL], F32)
            acc = cpool.tile([128, NCOL], F32)

            # Optional HAM warm-up: dummy matmuls on the (already loaded)
            # oodt tile while the first rt DMA is in flight. These have no
            # data deps on rt, so Tile schedules them first; they warm the
            # PE clock gate so the real stream runs at 2.4 GHz sooner.
            for w in range(warmup):
                wps = ppool.tile([128, 1024], F32, tag="ps")
                nc.tensor.matmul(wps[:, 0:512],
                                 oodt_s[:, 0:128], oodt_s[:, 0:512],
                                 start=True, stop=True)

            for k in range(KPC):
                rtk = rtpool.tile([128, NJ * D], store_dt)
                if k < rt_split_labels:
                    # split the load so this label's first matmuls can
                    # start after ~1/6 of the transfer
                    for jc in range(NJ):
                        nc.sync.dma_start(
                            out=rtk[:, jc * D:(jc + 1) * D],
                            in_=rt[k, :, jc * D:(jc + 1) * D])
                else:
                    nc.sync.dma_start(out=rtk[:], in_=rt[k, :, :])
                # oodc = oodT - c_k (c_k is a per-partition scalar per jc)
                oodc = spool.tile([128, NJ * B], store_dt, tag="oodc")
                for jc in range(NJ):
                    nc.vector.tensor_scalar_sub(
                        oodc[:, jc * B:(jc + 1) * B],
                        oodt_s[:, jc * B:(jc + 1) * B],
                        ck_s[:, k * NJ + jc:k * NJ + jc + 1])
                for bc in range(NBC):
                    ps = ppool.tile([128, 1024], F32, tag="ps")
                    for jc in range(NJ):
                        if bc < 4:
                            lhsT = oodc[:, jc * B + bc * 128:
                                        jc * B + (bc + 1) * 128]
                        else:
                            lhsT = xpost_s[:, jc * 128:(jc + 1) * 128]
                        nc.tensor.matmul(
                            ps[:, 0:512], mmcast(lhsT),
                            mmcast(rtk[:, jc * D:jc * D + 512]),
                            start=(jc == 0), stop=(jc == NJ - 1))
                        nc.tensor.matmul(
                            ps[:, 512:768], mmcast(lhsT),
                            mmcast(rtk[:, jc * D + 512:(jc + 1) * D]),
                            start=(jc == 0), stop=(jc == NJ - 1))
                    sq = spool.tile([128, D], F32)
                    col = k * NBC + bc
                    nc.scalar.activation(sq[:], ps[:, 0:768], AF.Square,
                                         accum_out=acc[:, col:col + 1])

            nc.sync.dma_start(out=dk_s[:], in_=dk[:])
            nc.sync.dma_start(out=om_s[:], in_=oodmask[:])
            nc.sync.dma_start(out=pm_s[:], in_=posmask[:])

            if debug_acc:
                nc.sync.dma_start(out=accdump[:], in_=acc[:])
            # ---- batched epilogue over the [128, 95] accumulator ----
            dist = fpool.tile([128, NCOL], F32)
            nc.scalar.activation(dist[:], acc[:], AF.Sqrt)
            t = fpool.tile([128, NCOL], F32)        # t = dist - dk
            nc.vector.tensor_tensor(out=t[:], in0=dist[:], in1=dk_s[:],
                                    op=ALU.subtract)
            relu_t = fpool.tile([128, NCOL], F32)   # (euc - d)+
            nc.scalar.activation(relu_t[:], t[:], AF.Relu)
            nrelu = fpool.tile([128, NCOL], F32)    # (d - euc)+
            nc.scalar.activation(nrelu[:], t[:], AF.Relu, scale=-1.0)
            m = fpool.tile([128, NCOL], F32)        # 1[d > euc]
            nc.scalar.activation(m[:], nrelu[:], AF.Sign)
            pnum_i = fpool.tile([128, NCOL], F32)   # 1[euc > d]
            nc.scalar.activation(pnum_i[:], relu_t[:], AF.Sign)
            e = fpool.tile([128, NCOL], F32)        # beta * exp(dk - dist)
            nc.scalar.activation(e[:], t[:], AF.Exp, scale=-1.0,
                                 bias=LNBETA)
            # e is only selected when dk <= dist, where e <= beta; clamp so
            # the branchless blend below can't catastrophically cancel.
            nc.vector.tensor_scalar_min(e[:], e[:], BETA)
            inb = fpool.tile([128, NCOL], F32)      # dk - dist + beta
            nc.scalar.activation(inb[:], t[:], AF.Copy, scale=-1.0,
                                 bias=BETA)
            # pl = e + m * (inb - e)   (branchless where(dk > dist, inb, e))
            d1 = fpool.tile([128, NCOL], F32)
            nc.vector.tensor_tensor(out=d1[:], in0=inb[:], in1=e[:],
                                    op=ALU.subtract)
            d2 = fpool.tile([128, NCOL], F32)
            nc.vector.tensor_tensor(out=d2[:], in0=m[:], in1=d1[:],
                                    op=ALU.mult)
            pl = fpool.tile([128, NCOL], F32)
            nc.vector.tensor_tensor(out=pl[:], in0=e[:], in1=d2[:],
                                    op=ALU.add)

            out4 = fpool.tile([128, 4], F32)
            for idx, (a, b) in enumerate([(pl, om_s), (relu_t, pm_s),
                                          (pnum_i, pm_s), (m, pm_s)]):
                tmp = fpool.tile([128, NCOL], F32, tag="redtmp")
                nc.vector.tensor_tensor(out=tmp[:], in0=a[:], in1=b[:],
                                        op=ALU.mult)
                nc.vector.tensor_reduce(out=out4[:, idx:idx + 1], in_=tmp[:],
                                        axis=mybir.AxisListType.X, op=ALU.add)
            nc.sync.dma_start(out=partials[:], in_=out4[:])

    nc.compile()
    return nc


SF = 2048.0     # fp8 scale for the off-diagonal matrix F = R - diag
SX = 16.0       # fp8 scale for the x operand
PSC = SF * SX   # 32768: psum carries PSC * rx
F8 = mybir.dt.float8e4
BF16 = mybir.dt.bfloat16
EPG = 5         # labels per epilogue batch


def _pin_act_table(patch=True):
    """Make the act-table chooser resolve every function to
    natural_log_exp_and_others (covers Square/Ln/Exp — the only ScalarE
    funcs this program uses), so exactly one ACT_TABLE_LOAD is emitted.
    Set ids are indices into act_info.json, so set order/length is kept
    and only the contents of the other sets are blanked."""
    if not patch:
        return
    orig = bacc.get_activation_tables

    def pinned(arch):
        tabs = orig(arch)
        keep = "natural_log_exp_and_others"
        if keep in tabs:
            for name in tabs:
                if name != keep:
                    tabs[name] = set()
        return tabs

    bacc.get_activation_tables = pinned


def build_program_dr8(warmup=10, rt_bufs=4):
    """fp8 DoubleRow program: psum = SF*SX*(F @ x8) + 32768*(x*D)^T."""
    _pin_act_table()
    nc = bacc.Bacc("TRN2", target_bir_lowering=False, debug=False,
                   num_devices=NCORES)

    rt = nc.dram_tensor("rt", [KPC, 128, NJ * D], F8,
                        kind="ExternalInput").ap()
    oodt = nc.dram_tensor("oodt", [128, NJ * B], BF16,
                          kind="ExternalInput").ap()
    ck = nc.dram_tensor("ck", [128, KPC * NJ], F32,
                        kind="ExternalInput").ap()
    dkd = nc.dram_tensor("dkd", [128, KPC * NJ], F32,
                         kind="ExternalInput").ap()
    xpost_q = nc.dram_tensor("xpost_q", [128, NJ * 128], F8,
                             kind="ExternalInput").ap()
    xpost_b = nc.dram_tensor("xpost_b", [128, NJ * 128], BF16,
                             kind="ExternalInput").ap()
    ident = nc.dram_tensor("ident", [128, 128], BF16,
                           kind="ExternalInput").ap()
    dk = nc.dram_tensor("dk", [128, NCOL], F32, kind="ExternalInput").ap()
    oodmask = nc.dram_tensor("oodmask", [128, NCOL], F32,
                             kind="ExternalInput").ap()
    posmask = nc.dram_tensor("posmask", [128, NCOL], F32,
                             kind="ExternalInput").ap()
    partials = nc.dram_tensor("partials", [128, 4], F32,
                              kind="ExternalOutput").ap()

    LNBETA = float(math.log(BETA))
    LNEPS = 1e-12
    const_t = nc.alloc_sbuf_tensor("const-lnbeta", [128, 1], F32)
    nc.gpsimd.memset(const_t.ap(), LNBETA)
    nc.const_aps.aps[(F32, LNBETA)] = const_t.ap()
    const_e = nc.alloc_sbuf_tensor("const-lneps", [128, 1], F32)
    nc.gpsimd.memset(const_e.ap(), LNEPS)
    nc.const_aps.aps[(F32, LNEPS)] = const_e.ap()
    nc.all_engine_barrier()

    AF = mybir.ActivationFunctionType
    ALU = mybir.AluOpType
    DR = mybir.MatmulPerfMode.DoubleRow
    DESCALE = float(1.0 / (PSC * PSC))   # 2^-30: Sqrt input pre-scale

    with tile.TileContext(nc) as tc:
        with (
            tc.tile_pool(name="consts", bufs=1) as cpool,
            tc.tile_pool(name="rtp", bufs=rt_bufs) as rtpool,
            tc.tile_pool(name="scratch", bufs=3) as spool,
            tc.tile_pool(name="psum", bufs=4, space="PSUM") as ppool,
            tc.tile_pool(name="fin", bufs=1) as fpool,
            tc.tile_pool(name="epi", bufs=2) as epool,
        ):
            # --- constants / shared loads, critical-path first ---
            ident_s = cpool.tile([128, 128], BF16)
            nc.sync.dma_start(out=ident_s[:], in_=ident[:])
            oodt_s = cpool.tile([128, NJ, B], BF16)
            for jc in range(NJ):
                nc.sync.dma_start(out=oodt_s[:, jc, :],
                                  in_=oodt[:, jc * B:(jc + 1) * B])
            ck_s = cpool.tile([128, KPC * NJ], F32)
            nc.sync.dma_start(out=ck_s[:], in_=ck[:])
            dkd_s = cpool.tile([128, KPC * NJ], F32)
            nc.sync.dma_start(out=dkd_s[:], in_=dkd[:])
            # needed only from bc==4 of label 0 / first epilogue: loaded
            # after the first rt labels so rt0 wins the DMA queues
            xq_s = cpool.tile([128, NJ, 128], F8)
            xb_s = cpool.tile([128, NJ, 128], BF16)
            dk_s = cpool.tile([128, NCOL], F32)
            om_s = cpool.tile([128, NCOL], F32)
            pm_s = cpool.tile([128, NCOL], F32)
            # per-epilogue-batch accumulators: avoids WAR serialization
            # between later squares and the batched epilogue reads.
            # Last batch is a single label so the serial tail chain is tiny.
            BSIZES = [5, 5, 5, 3, 1]
            assert sum(BSIZES) == KPC
            BSTART, BATCH_OF, BEND = [], {}, set()
            s = 0
            for g, bsz in enumerate(BSIZES):
                BSTART.append(s)
                for kk in range(s, s + bsz):
                    BATCH_OF[kk] = g
                BEND.add(s + bsz - 1)
                s += bsz
            accs = [cpool.tile([128, bsz * NBC], F32, name=f"accb{g}")
                    for g, bsz in enumerate(BSIZES)]
            st_pl = cpool.tile([128, NCOL], F32)
            st_rl = cpool.tile([128, NCOL], F32)
            st_pn = cpool.tile([128, NCOL], F32)
            st_m = cpool.tile([128, NCOL], F32)

            # HAM warm-up on a memset tile: no DMA dependency, starts the
            # PE activity window as early as possible.
            dummy = cpool.tile([128, 384], BF16)
            nc.vector.memset(dummy[:], 1.0)
            for w in range(warmup):
                wps = ppool.tile([128, 1024], F32, tag="ps")
                nc.tensor.matmul(wps[:, 0:384], dummy[:, 0:128], dummy[:],
                                 start=True, stop=True)

            for k in range(KPC):
                rtk = rtpool.tile([128, NJ, D], F8)
                for pr in range(NJ // 2):
                    nc.sync.dma_start(
                        out=rtk[:, 2 * pr:2 * pr + 2, :],
                        in_=rt[k, :, 2 * pr * D:(2 * pr + 2) * D])
                if k == 0:
                    # after rt0 hits the queues; needed from bc==4 of k==0
                    nc.sync.dma_start(out=xq_s[:], in_=xpost_q[:])
                    nc.sync.dma_start(out=xb_s[:], in_=xpost_b[:])
                elif k == 1:
                    # needed first at the label-4 epilogue
                    nc.sync.dma_start(out=dk_s[:], in_=dk[:])
                    nc.sync.dma_start(out=om_s[:], in_=oodmask[:])
                    nc.sync.dma_start(out=pm_s[:], in_=posmask[:])
                # x8 = e4m3(SX*ood - SX*ck)  per jc chunk (fp8 stationary)
                oq = spool.tile([128, NJ, B], F8, tag="oq")
                xd = spool.tile([128, NJ, NBC * 128], BF16, tag="xd")
                for jc in range(NJ):
                    c1 = ck_s[:, k * NJ + jc:k * NJ + jc + 1]
                    d1 = dkd_s[:, k * NJ + jc:k * NJ + jc + 1]
                    nc.vector.tensor_scalar(
                        out=oq[:, jc, :], in0=oodt_s[:, jc, :],
                        scalar1=c1, scalar2=None, op0=ALU.subtract)
                    # xd = (SX*x) * (D/SX) = x*D  (exact-x diagonal path)
                    nc.vector.tensor_scalar(
                        out=xd[:, jc, 0:B], in0=oodt_s[:, jc, :],
                        scalar1=c1, scalar2=d1,
                        op0=ALU.subtract, op1=ALU.mult)
                    nc.vector.tensor_scalar(
                        out=xd[:, jc, B:B + 128], in0=xb_s[:, jc, :],
                        scalar1=d1, scalar2=None, op0=ALU.mult)
                for bc in range(NBC):
                    ps = ppool.tile([128, 1024], F32, tag="ps")
                    for pr in range(NJ // 2):
                        if bc < 4:
                            lhsT = oq[:, 2 * pr:2 * pr + 2,
                                      bc * 128:(bc + 1) * 128]
                        else:
                            lhsT = xq_s[:, 2 * pr:2 * pr + 2, :]
                        nc.tensor.matmul(
                            ps[:, 0:512], lhsT, rtk[:, 2 * pr:2 * pr + 2, 0:512],
                            start=(pr == 0), stop=False, perf_mode=DR,
                            skip_group_check=True)
                        nc.tensor.matmul(
                            ps[:, 512:768], lhsT,
                            rtk[:, 2 * pr:2 * pr + 2, 512:768],
                            start=(pr == 0), stop=False, perf_mode=DR,
                            skip_group_check=True)
                    # exact diagonal: ps[:, jc*128:(jc+1)*128] += 32768*xd^T
                    for jc in range(NJ):
                        lh2 = xd[:, jc, bc * 128:(bc + 1) * 128]
                        nc.tensor.matmul(
                            ps[:, jc * 128:(jc + 1) * 128], lh2, ident_s[:],
                            start=False, stop=(jc == 3 or jc == NJ - 1),
                            skip_group_check=True)
                    sq = spool.tile([128, D], F32, tag="sq")
                    g = BATCH_OF[k]
                    col = (k - BSTART[g]) * NBC + bc
                    nc.scalar.activation(sq[:], ps[:, 0:768], AF.Square,
                                         accum_out=accs[g][:, col:col + 1])

                # --- incremental epilogue once a batch of labels is done ---
                # ScalarE uses only {Ln, Exp, Square}: one act-table set
                # (natural_log_exp_and_others), so no table reloads.
                if k in BEND:
                    g = BATCH_OF[k]
                    g0 = BSTART[g] * NBC
                    w = (k + 1 - BSTART[g]) * NBC
                    sl = slice(g0, g0 + w)
                    # dist = exp(0.5*ln(acc*DESCALE + eps)) == sqrt
                    lg = epool.tile([128, w], F32, tag="e0")
                    nc.scalar.activation(lg[:], accs[g][:], AF.Ln,
                                         scale=DESCALE, bias=1e-12)
                    dist = epool.tile([128, w], F32, tag="e11")
                    nc.scalar.activation(dist[:], lg[:], AF.Exp, scale=0.5)
                    t = epool.tile([128, w], F32, tag="e1")
                    nc.vector.tensor_tensor(out=t[:], in0=dist[:],
                                            in1=dk_s[:, sl], op=ALU.subtract)
                    relu_t = epool.tile([128, w], F32, tag="e2")
                    nc.vector.tensor_scalar_max(relu_t[:], t[:], 0.0)
                    m = epool.tile([128, w], F32, tag="e4")
                    nc.vector.tensor_scalar(out=m[:], in0=t[:], scalar1=0.0,
                                            scalar2=None, op0=ALU.is_lt)
                    pnum_i = epool.tile([128, w], F32, tag="e5")
                    nc.vector.tensor_scalar(out=pnum_i[:], in0=t[:],
                                            scalar1=0.0, scalar2=None,
                                            op0=ALU.is_gt)
                    e = epool.tile([128, w], F32, tag="e6")
                    nc.scalar.activation(e[:], t[:], AF.Exp, scale=-1.0,
                                         bias=LNBETA)
                    nc.vector.tensor_scalar_min(e[:], e[:], BETA)
                    inb = epool.tile([128, w], F32, tag="e7")
                    nc.vector.tensor_scalar(out=inb[:], in0=t[:],
                                            scalar1=-1.0, scalar2=BETA,
                                            op0=ALU.mult, op1=ALU.add)
                    d1t = epool.tile([128, w], F32, tag="e8")
                    nc.vector.tensor_tensor(out=d1t[:], in0=inb[:], in1=e[:],
                                            op=ALU.subtract)
                    d2t = epool.tile([128, w], F32, tag="e9")
                    nc.vector.tensor_tensor(out=d2t[:], in0=m[:], in1=d1t[:],
                                            op=ALU.mult)
                    pl = epool.tile([128, w], F32, tag="e10")
                    nc.vector.tensor_tensor(out=pl[:], in0=e[:], in1=d2t[:],
                                            op=ALU.add)
                    nc.vector.tensor_tensor(out=st_pl[:, sl], in0=pl[:],
                                            in1=om_s[:, sl], op=ALU.mult)
                    nc.vector.tensor_tensor(out=st_rl[:, sl], in0=relu_t[:],
                                            in1=pm_s[:, sl], op=ALU.mult)
                    nc.vector.tensor_tensor(out=st_pn[:, sl], in0=pnum_i[:],
                                            in1=pm_s[:, sl], op=ALU.mult)
                    nc.vector.tensor_tensor(out=st_m[:, sl], in0=m[:],
                                            in1=pm_s[:, sl], op=ALU.mult)

            out4 = fpool.tile([128, 4], F32)
            for idx, a in enumerate([st_pl, st_rl, st_pn, st_m]):
                nc.vector.tensor_reduce(out=out4[:, idx:idx + 1], in_=a[:],
                                        axis=mybir.AxisListType.X, op=ALU.add)
            nc.sync.dma_start(out=partials[:], in_=out4[:])

    nc.compile()
    return nc


def prep_inputs_dr8(pooled_output, centroids, delta, L, U, D_diag, ood,
                    labels):
    """Host-side shard prep for the dr8 program."""
    import ml_dtypes
    E4NP = ml_dtypes.float8_e4m3
    BFNP = ml_dtypes.bfloat16

    pooled_output = np.asarray(pooled_output, np.float32)
    centroids = np.asarray(centroids, np.float32)
    delta = np.asarray(delta, np.float32)
    L = np.asarray(L, np.float32)
    U = np.asarray(U, np.float32)
    D_diag = np.asarray(D_diag, np.float32)
    ood = np.asarray(ood, np.float32)
    labels = np.asarray(labels).astype(np.int64)

    # F^T = (R - diag)^T in DMA layout [k, p, jc*768 + i], scaled by SF.
    rows, cols = np.tril_indices(D, -1)
    rt_all = np.zeros((KPAD, 128, NJ * D), np.float32)
    rt_all[:K, rows % 128, (rows // 128) * D + cols] = U
    rt_all[:K, cols % 128, (cols // 128) * D + rows] = L
    rt_all *= SF
    rt8 = rt_all.astype(E4NP)
    del rt_all

    def pack_cols(mat):  # [768, n] -> [128, NJ*n] in (p, (jc, n)) layout
        n = mat.shape[1]
        return (mat.reshape(NJ, 128, n).transpose(1, 0, 2)
                .reshape(128, NJ * n))

    oodt_h = pack_cols(ood.T * SX).astype(BFNP)

    delta_pad = np.zeros(KPAD, np.float32)
    delta_pad[:K] = delta
    cent_pad = np.zeros((KPAD, D), np.float32)
    cent_pad[:K] = centroids
    ddiag_pad = np.zeros((KPAD, D), np.float32)
    ddiag_pad[:K] = D_diag

    ident_h = (np.eye(128, dtype=np.float32) * PSC).astype(BFNP)

    in_maps = []
    for mcore in range(NCORES):
        k0 = mcore * KPC
        sel = np.where((labels >= k0) & (labels < k0 + KPC))[0]
        sel = sel[np.argsort(labels[sel], kind="stable")]
        n_pos = len(sel)
        assert n_pos <= 128, f"core {mcore} has {n_pos} positive samples"

        xpos = np.zeros((D, 128), np.float32)
        slot_label = np.full(128, -1, np.int64)
        if n_pos:
            xpos[:, :n_pos] = (pooled_output[sel] - centroids[labels[sel]]).T
            slot_label[:n_pos] = labels[sel]
        xpos_sc = pack_cols(xpos * SX)

        dk_t = np.zeros((128, NCOL), np.float32)
        om_t = np.zeros((128, NCOL), np.float32)
        pm_t = np.zeros((128, NCOL), np.float32)
        for kl in range(KPC):
            kg = k0 + kl
            dk_t[:, kl * NBC:(kl + 1) * NBC] = delta_pad[kg]
            if kg < K:
                om_t[:, kl * NBC:kl * NBC + 4] = 1.0
                pm_t[:, kl * NBC + 4] = (slot_label == kg).astype(np.float32)

        ck_t = (cent_pad[k0:k0 + KPC].reshape(KPC, NJ, 128)
                .transpose(2, 0, 1).reshape(128, KPC * NJ)) * SX
        dkd_t = (ddiag_pad[k0:k0 + KPC].reshape(KPC, NJ, 128)
                 .transpose(2, 0, 1).reshape(128, KPC * NJ)) / SX

        in_maps.append({
            "rt": rt8[k0:k0 + KPC],
            "oodt": oodt_h,
            "ck": ck_t.astype(np.float32),
            "dkd": dkd_t.astype(np.float32),
            "xpost_q": xpos_sc.astype(E4NP),
            "xpost_b": xpos_sc.astype(BFNP),
            "ident": ident_h,
            "dk": dk_t,
            "oodmask": om_t,
            "posmask": pm_t,
        })
    return in_maps


def prep_inputs(pooled_output, centroids, delta, L, U, D_diag, ood, labels,
                mode="f32r"):
    """Host-side shard prep. Returns in_maps (list of 8 dicts)."""
    pooled_output = np.asarray(pooled_output, np.float32)
    centroids = np.asarray(centroids, np.float32)
    delta = np.asarray(delta, np.float32)
    L = np.asarray(L, np.float32)
    U = np.asarray(U, np.float32)
    D_diag = np.asarray(D_diag, np.float32)
    ood = np.asarray(ood, np.float32)
    labels = np.asarray(labels).astype(np.int64)

    store_np = np.float32
    if mode == "bf16":
        import ml_dtypes
        store_np = ml_dtypes.bfloat16

    # RT[k] = R[k].T built directly in the DMA layout [k, p, jc*768 + i]:
    # element (row a, col b) of R.T lives at partition a%128, free (a//128)*768+b.
    rows, cols = np.tril_indices(D, -1)
    rt_all = np.zeros((KPAD, 128, NJ * D), np.float32)
    # strict lower of R.T (a=rows, b=cols) holds U
    rt_all[:K, rows % 128, (rows // 128) * D + cols] = U
    # strict upper of R.T (a=cols, b=rows) holds L
    rt_all[:K, cols % 128, (cols // 128) * D + rows] = L
    dia = np.arange(D)
    rt_all[:K, dia % 128, (dia // 128) * D + dia] = D_diag

    def pack_cols(mat):  # [768, n] -> [128, NJ*n] in (p, (jc, n)) layout
        n = mat.shape[1]
        return (mat.reshape(NJ, 128, n).transpose(1, 0, 2)
                .reshape(128, NJ * n).astype(store_np))

    oodt_h = pack_cols(ood.T.astype(np.float32))

    delta_pad = np.zeros(KPAD, np.float32)
    delta_pad[:K] = delta
    cent_pad = np.zeros((KPAD, D), np.float32)
    cent_pad[:K] = centroids

    in_maps = []
    for mcore in range(NCORES):
        k0 = mcore * KPC
        lab_lo, lab_hi = k0, k0 + KPC
        sel = np.where((labels >= lab_lo) & (labels < lab_hi))[0]
        sel = sel[np.argsort(labels[sel], kind="stable")]
        n_pos = len(sel)
        assert n_pos <= 128, f"core {mcore} has {n_pos} positive samples"

        xpos = np.zeros((D, 128), np.float32)
        slot_label = np.full(128, -1, np.int64)
        if n_pos:
            xpos[:, :n_pos] = (pooled_output[sel] - centroids[labels[sel]]).T
            slot_label[:n_pos] = labels[sel]

        dk_t = np.zeros((128, NCOL), np.float32)
        om_t = np.zeros((128, NCOL), np.float32)
        pm_t = np.zeros((128, NCOL), np.float32)
        for kl in range(KPC):
            kg = k0 + kl
            dk_t[:, kl * NBC:(kl + 1) * NBC] = delta_pad[kg]
            if kg < K:
                om_t[:, kl * NBC:kl * NBC + 4] = 1.0
                pm_t[:, kl * NBC + 4] = (slot_label == kg).astype(np.float32)

        ck_t = (cent_pad[k0:k0 + KPC].reshape(KPC, NJ, 128)
                .transpose(2, 0, 1).reshape(128, KPC * NJ).astype(store_np))

        rt_m = rt_all[k0:k0 + KPC]
        if rt_m.dtype != store_np:
            rt_m = rt_m.astype(store_np)
        in_maps.append({
            "rt": rt_m,
            "oodt": oodt_h,
            "ck": ck_t,
            "xpost": pack_cols(xpos),
            "dk": dk_t,
            "oodmask": om_t,
            "posmask": pm_t,
        })
    return in_maps


def combine(results):
    """Host-side reduction of per-core [128, 4] partials to the 5 outputs."""
    tot = np.zeros(4, np.float64)
    for r in results:
        tot += np.asarray(r["partials"], np.float64).sum(axis=0)
    neg_sum, pos_sum, pos_num, neg_num = tot
    pos_mean = pos_sum / B
    neg_mean = neg_sum / B
    return (np.float32(pos_mean), np.float32(neg_mean),
            np.float32(pos_num), np.float32(neg_num),
            np.float32(pos_mean + neg_mean))


def kernel(pooled_output, centroids, delta, L, U, D_diag, ood, labels,
           mode="dr8", trace=False):
    if mode not in _prog_cache:
        if mode == "dr8":
            _prog_cache[mode] = build_program_dr8()
        else:
            _prog_cache[mode] = build_program(mode)
    nc = _prog_cache[mode]
    if mode == "dr8":
        in_maps = prep_inputs_dr8(pooled_output, centroids, delta, L, U,
                                  D_diag, ood, labels)
    else:
        in_maps = prep_inputs(pooled_output, centroids, delta, L, U, D_diag,
                              ood, labels, mode=mode)
    res = run_bass_kernel_spmd(nc, in_maps, list(range(NCORES)), trace=trace)
    out = combine(res.results)
    if trace:
        return out, res
    return out

